# revision 1
# baseline (speedup 1.0000x reference)
"""Sparse attention kernel for Trainium2 (8 NeuronCores, data-parallel over batch).

Reference computation (per batch row b):
    q    = x @ q_w.T                                  [N, C]
    xkv  = x[key_ind]                                 [NKV, C]
    kv   = xkv @ kv_w.T -> per-head k, v              [NKV, 2C]
    attn = softmax((q*scale) @ k.T) @ v               [N, C]
    out  = attn @ proj_w.T + proj_b                   [N, C]

Per-core layout strategy (core = one batch row):
  - Everything computed transposed ("feature on partition"): qT [C, N],
    kT/vT via kv_w.T, attention scores ST [m, n] so that softmax needs no
    transposes: P = exp(ST) is directly the PV stationary operand, and the
    softmax denominator falls out of an appended ones-column in v.
  - f32r (TF32-like) matmuls throughout: full PE rate at ~1e-3 rel err.
  - KV gather on device via indirect DMA + PE transposes.
"""
import os
import sys

sys.path.insert(0, "/opt/trn_rl_repo")

STAGE = int(os.environ.get("BK_STAGE", "99"))

import numpy as np  # noqa: E402

B, N, C = 8, 2048, 768
NKV = 512
H = 12
HD = C // H          # 64
SCALE = HD ** -0.5
P = 128
CT = C // P          # 6 feature tiles
NC2 = 512            # token chunk
NCH = N // NC2       # 4 chunks
MCH = NKV // P       # 4 key chunks
G = H // 2           # 6 head pairs

_CACHE = {}


def _build():
    import concourse.bass as bass
    import concourse.mybir as mybir
    import concourse.tile as tile
    from concourse import bacc
    from concourse.masks import make_identity
    from contextlib import ExitStack

    F32 = mybir.dt.float32
    F32R = mybir.dt.float32r
    I32 = mybir.dt.int32
    Exp = mybir.ActivationFunctionType.Exp
    Ident = mybir.ActivationFunctionType.Identity

    nc = bacc.Bacc("TRN2", target_bir_lowering=False, debug=False, num_devices=8)

    xT = nc.dram_tensor("xT", [CT, P, N], F32R, kind="ExternalInput")
    xr = nc.dram_tensor("xr", [N, C], F32, kind="ExternalInput")
    idx = nc.dram_tensor("idx", [P, MCH], I32, kind="ExternalInput")
    qwT = nc.dram_tensor("qwT", [CT, P, C], F32R, kind="ExternalInput")
    kwT = nc.dram_tensor("kwT", [CT, P, C], F32R, kind="ExternalInput")
    vwT = nc.dram_tensor("vwT", [CT, P, C], F32R, kind="ExternalInput")
    ones_in = nc.dram_tensor("ones_in", [P, HD], F32R, kind="ExternalInput")
    pwT = nc.dram_tensor("pwT", [CT, P, C], F32R, kind="ExternalInput")
    pb = nc.dram_tensor("pb", [P, CT], F32, kind="ExternalInput")
    out = nc.dram_tensor("out", [CT, P, N], F32, kind="ExternalOutput")

    with tile.TileContext(nc) as tc, ExitStack() as top:
        const = top.enter_context(tc.tile_pool(name="const", bufs=1))
        work = top.enter_context(tc.tile_pool(name="work", bufs=3))
        apool = top.enter_context(tc.tile_pool(name="apool", bufs=1))
        w3 = top.enter_context(tc.tile_pool(name="w3", bufs=2))
        ptp = top.enter_context(tc.tile_pool(name="ptp", bufs=5))
        rcrb = top.enter_context(tc.tile_pool(name="rcrb", bufs=1))
        w4 = top.enter_context(tc.tile_pool(name="w4", bufs=3))

        # ---------- persistent loads ----------
        idx_sb = const.tile([P, MCH], I32, tag="idx")
        nc.sync.dma_start(idx_sb[:], idx[:])
        qwT_sb = []
        kwT_sb = []
        vwT_sb = []
        pwT_sb = []
        pb_sb = const.tile([P, CT], F32, tag="pb")
        nc.sync.dma_start(pb_sb[:], pb[:])

        # ---------- gather + transpose + KV projection ----------
        kT_sb = []      # per head pair g: [128, NKV], rows 0-63 head 2g, 64-127 head 2g+1
        vaug_sb = []    # per m-chunk: [128, H*(HD+1)] f32r, col HD of each head block = 1.0
        with ExitStack() as ph:
            gpool = ph.enter_context(tc.tile_pool(name="gather", bufs=1))
            ps_tr = ph.enter_context(tc.tile_pool(name="ps_tr", bufs=4, space="PSUM"))
            ps_kv = ph.enter_context(tc.tile_pool(name="ps_kv", bufs=4, space="PSUM"))

            for i in range(CT):
                t = gpool.tile([P, C], F32R, tag=f"kwT{i}")
                nc.sync.dma_start(t[:], kwT[i, :, :])
                kwT_sb.append(t)
                t = gpool.tile([P, C], F32R, tag=f"vwT{i}")
                nc.sync.dma_start(t[:], vwT[i, :, :])
                vwT_sb.append(t)
            xkvT = []
            for i in range(CT):
                xkvT_i = gpool.tile([P, NKV], F32R, tag=f"xkvT{i}")
                xkvT.append(xkvT_i)
            xkv_tiles = []
            gather_insts = []
            for k in range(MCH):
                xkv = gpool.tile([P, C], F32, tag=f"xkv{k % 2}")
                gi = nc.gpsimd.indirect_dma_start(
                    out=xkv[:], out_offset=None, in_=xr[:],
                    in_offset=bass.IndirectOffsetOnAxis(ap=idx_sb[:, k:k + 1], axis=0))
                gather_insts.append(gi)
                xkv_tiles.append(xkv)
            ident = const.tile([P, P], F32, tag="ident")
            make_identity(nc, ident[:])
            for k in range(MCH):
                xkv = xkv_tiles[k]
                for i in range(CT):
                    tr = ps_tr.tile([P, P], F32, tag="tr")
                    nc.tensor.transpose(tr[:], xkv[:, i * P:(i + 1) * P], ident[:])
                    nc.scalar.copy(xkvT[i][:, k * P:(k + 1) * P], tr[:])

            # kT: per head pair
            for g in range(G):
                kp = ps_kv.tile([P, NKV], F32, tag="kv")
                for i in range(CT):
                    nc.tensor.matmul(kp[:], kwT_sb[i][:, g * P:(g + 1) * P], xkvT[i][:],
                                     start=(i == 0), stop=(i == CT - 1))
                kt = const.tile([P, NKV], F32R, tag=f"kT{g}")
                nc.vector.tensor_copy(kt[:], kp[:])
                kT_sb.append(kt)

            # v (+ ones col): [m, head*(HD+1)]
            for k in range(MCH):
                va = const.tile([P, H * (HD + 1)], F32R, tag=f"vaug{k}")
                va3 = va[:].rearrange("p (h x) -> p h x", x=HD + 1)
                for half in range(2):
                    vp = ps_kv.tile([P, 6 * HD], F32, tag="kv")
                    for i in range(CT):
                        nc.tensor.matmul(vp[:], xkvT[i][:, k * P:(k + 1) * P],
                                         vwT_sb[i][:, half * 6 * HD:(half + 1) * 6 * HD],
                                         start=(i == 0), stop=(i == CT - 1))
                    nc.scalar.copy(va3[:, 6 * half:6 * half + 6, 0:HD],
                                   vp[:].rearrange("p (h x) -> p h x", x=HD))
                nc.sync.dma_start(va3[:, :, HD:HD + 1], ones_in[:, 0:H])
                vaug_sb.append(va)

        # late weight loads (q/proj not needed until after gather/kv phase)
        for i in range(CT):
            t = const.tile([P, C], F32R, tag=f"qwT{i}")
            nc.sync.dma_start(t[:], qwT[i, :, :])
            qwT_sb.append(t)
            t = const.tile([P, C], F32R, tag=f"pwT{i}")
            nc.sync.dma_start(t[:], pwT[i, :, :])
            pwT_sb.append(t)

        # ---------- main loop over token chunks ----------
        if STAGE <= 1:
            z = work.tile([P, CT * NC2], F32, tag="outc")
            nc.gpsimd.memset(z[:], 0.0)
            for ch in range(NCH):
                sl = slice(ch * NC2, (ch + 1) * NC2)
                nc.sync.dma_start(out[:, :, sl].rearrange("i p n -> p i n"),
                                  z[:].rearrange("p (i n) -> p i n", i=CT))
            nc.compile()
            return nc
        ps_mm = top.enter_context(tc.tile_pool(name="ps_mm", bufs=2, space="PSUM"))
        ps_st = top.enter_context(tc.tile_pool(name="ps_st", bufs=2, space="PSUM"))
        ps_ov = top.enter_context(tc.tile_pool(name="ps_ov", bufs=2, space="PSUM"))
        for ch in range(NCH):
            sl = slice(ch * NC2, (ch + 1) * NC2)
            xTc = []
            for i in range(CT):
                t = w4.tile([P, NC2], F32R, tag=f"xTc{i}")
                nc.sync.dma_start(t[:], xT[i, :, sl])
                xTc.append(t)

            # qT for this chunk: per head pair g -> [128, NC2]
            qT = []
            for j in range(CT):
                qp = ps_mm.tile([P, NC2], F32, tag="mm")
                for i in range(CT):
                    nc.tensor.matmul(qp[:], qwT_sb[i][:, j * P:(j + 1) * P], xTc[i][:],
                                     start=(i == 0), stop=(i == CT - 1))
                qt = w3.tile([P, NC2], F32R, tag=f"qT{j}")
                nc.vector.tensor_copy(qt[:], qp[:])
                qT.append(qt)

            # attention per head; output packed per head pair:
            # attn[g] [128, NC2], rows 0-63 = head 2g, rows 64-127 = head 2g+1
            attn = []
            for g in range(G):
                at = apool.tile([P, NC2], F32R, tag=f"attn{g}")
                # all 8 STs of the pair adjacent: T0/T8 row-group streams
                # overlap on the PE and the 64-row mode is entered once
                pts2 = {}
                for par in range(2):
                    base = par * HD
                    for k in range(MCH):
                        st = ps_st.tile([P, NC2], F32, tag="st")
                        nc.tensor.matmul(
                            st[:],
                            kT_sb[g][base:base + HD, k * P:(k + 1) * P],
                            qT[g][base:base + HD, :],
                            start=True, stop=True)
                        pt = ptp.tile([P, NC2], F32R, tag="pt")
                        nc.scalar.activation(pt[:], st[:], Exp, scale=SCALE)
                        pts2[(par, k)] = pt
                for par in range(2):
                    h = 2 * g + par
                    base = par * HD
                    ov = ps_ov.tile([HD + 1, NC2], F32, tag="ov")
                    for k in range(MCH):
                        nc.tensor.matmul(ov[:], vaug_sb[k][:, h * (HD + 1):(h + 1) * (HD + 1)],
                                         pts2[(par, k)][:], start=(k == 0),
                                         stop=(k == MCH - 1))
                    rc = rcrb.tile([1, NC2], F32, tag="rc")
                    nc.vector.reciprocal(rc[:], ov[HD:HD + 1, :])
                    rb = rcrb.tile([HD, NC2], F32, tag="rb")
                    nc.gpsimd.partition_broadcast(rb[:], rc[:])
                    nc.vector.tensor_mul(at[base:base + HD, :], ov[0:HD, :], rb[:])
                attn.append(at)

            # output projection + bias (plain K=128 over head pairs)
            for j in range(CT):
                pp = ps_mm.tile([P, NC2], F32, tag="mmp")
                for i in range(CT):
                    nc.tensor.matmul(
                        pp[:], pwT_sb[i][:, j * P:(j + 1) * P], attn[i][:],
                        start=(i == 0), stop=(i == CT - 1))
                oj = work.tile([P, NC2], F32, tag="oj")
                nc.vector.tensor_scalar_add(oj[:], pp[:], pb_sb[:, j:j + 1])
                nc.sync.dma_start(out[j, :, sl], oj[:])

    nc.compile()
    return nc


def _get_nc():
    if "nc" not in _CACHE:
        _CACHE["nc"] = _build()
    return _CACHE["nc"]


def _prep_core_inputs(x, key_ind, q_w, kv_w, proj_w, proj_b):
    """Build the 8 per-core input maps."""
    x = np.ascontiguousarray(x, dtype=np.float32)
    qwT = np.ascontiguousarray(q_w.T.astype(np.float32).reshape(CT, P, C))
    kvwT = kv_w.T.astype(np.float32)                       # [C, 2C]
    kvwT3 = kvwT.reshape(C, H, 2 * HD)
    kwT = np.ascontiguousarray(kvwT3[:, :, :HD].reshape(CT, P, C))
    vwT = np.ascontiguousarray(kvwT3[:, :, HD:].reshape(CT, P, C))
    ones_hd = np.ones((P, HD), dtype=np.float32)
    pwT = np.ascontiguousarray(proj_w.T.astype(np.float32).reshape(CT, P, C))
    pb = np.ascontiguousarray(proj_b.astype(np.float32).reshape(CT, P).T)
    in_maps = []
    for b in range(B):
        xb = x[b]                                   # [N, C]
        xTb = np.ascontiguousarray(xb.T.reshape(CT, P, N))
        idxb = np.ascontiguousarray(
            key_ind[b].astype(np.int32).reshape(MCH, P).T)
        in_maps.append({
            "xT": xTb, "xr": xb, "idx": idxb,
            "qwT": qwT, "kwT": kwT, "vwT": vwT, "pwT": pwT, "pb": pb,
            "ones_in": ones_hd,
        })
    return in_maps


def kernel(x, key_ind, q_w, kv_w, proj_w, proj_b, _trace=False, _results=None):
    from concourse.bass_utils import run_bass_kernel_spmd

    nc = _get_nc()
    in_maps = _prep_core_inputs(x, key_ind, q_w, kv_w, proj_w, proj_b)
    res = run_bass_kernel_spmd(nc, in_maps, core_ids=list(range(B)), trace=_trace)
    if _results is not None:
        _results.append(res)
    out = np.empty((B, N, C), dtype=np.float32)
    for b in range(B):
        out[b] = res.results[b]["out"].reshape(C, N).T
    return out

